# revision 1
# baseline (speedup 1.0000x reference)
"""Multi-head attention (B=2,S=4096,E=768,H=12,D=64 + 16-token K/V prompt
prefix) on 8 Trainium2 NeuronCores.

Sharding: 2 batches x 4 head-groups (3 heads each). Each core computes QKV
projections for its 3 heads, full attention over its batch, and a partial
output projection (its 192 ctx channels); the host sums the 4 partials per
batch.

Per-core kernel layout (all attention math in "transposed" orientation so no
on-chip transposes are needed):
  qT[c,s]   = Wq_g @ query^T           (lhsT=Wq_g^T chunks, rhs=queryT chunks)
  kT[c,s]   likewise; prompt K prefix DMA'd in pre-transposed from host
  v[s,c]    natural orientation        (lhsT=valueT chunks, rhs=Wv_g^T)
  scoresT[k,q] = kT^T-slices @ qT      (lhsT=kT tile [64,128], rhs=qT [64,512])
  expT = Exp(scoresT / sqrt(D))        (ScalarE, reads PSUM directly)
  ctxT[d,q](+denom row) = v_aug^T @ expT  (v_aug has a ones column -> row 64
                                           accumulates the softmax denominator)
  ctxT_norm = ctxT * bcast(1/denom)    (fused into PSUM evacuation)
  outT[e,q] partial = Wo_g^T-slices @ ctxT_norm

Pipelining: one global software-pipelined slot stream over (sq, h, kt).
Scores matmuls + exp lead; ctx matmuls trail by TRAIL slots; V-projection
matmuls ride the first 32 slots; the partial out-projection for a q-block is
emitted as soon as its last head is normalized.  ScalarE (exp) is the
bottleneck engine and is kept ~100% busy.
"""

import sys
import threading

import numpy as np

if "/opt/trn_rl_repo" not in sys.path:
    sys.path.insert(0, "/opt/trn_rl_repo")

import ml_dtypes

BF16 = ml_dtypes.bfloat16

B, S, E, H, D, PP = 2, 4096, 768, 12, 64, 16
NCORES = 8
NG = 4          # head-groups (tensor parallel)
HL = H // NG    # 3 local heads
CL = HL * D     # 192 local channels
SKV = PP + S    # 4112
NKT = S // 128  # 32 full k-tiles (prefix handled separately)
QT = 1024       # q tile width for scores/exp/ctx
NSQ = S // QT   # 4
TRAIL = 12       # ctx matmuls trail scores by this many slots
NST = S // 128  # 32 v stiles

_lock = threading.Lock()
_compiled = {}


def _build():
    import concourse.bass as bass  # noqa: F401
    import concourse.mybir as mybir
    import concourse.tile as tile
    from concourse import bacc

    f32 = mybir.dt.float32
    bf16 = mybir.dt.bfloat16
    EXP = mybir.ActivationFunctionType.Exp

    nc = bacc.Bacc("TRN2", target_bir_lowering=False, debug=False)

    xqT = nc.dram_tensor("xqT", [E, S], bf16, kind="ExternalInput").ap()
    xkT = nc.dram_tensor("xkT", [E, S], bf16, kind="ExternalInput").ap()
    xvT = nc.dram_tensor("xvT", [E, S], bf16, kind="ExternalInput").ap()
    wqT = nc.dram_tensor("wqT", [E, CL], bf16, kind="ExternalInput").ap()
    wkT = nc.dram_tensor("wkT", [E, CL], bf16, kind="ExternalInput").ap()
    wvT = nc.dram_tensor("wvT", [E, CL], bf16, kind="ExternalInput").ap()
    woT = nc.dram_tensor("woT", [CL, E], bf16, kind="ExternalInput").ap()
    bq = nc.dram_tensor("bq", [CL, 1], f32, kind="ExternalInput").ap()
    bk = nc.dram_tensor("bk", [CL, 1], f32, kind="ExternalInput").ap()
    bv = nc.dram_tensor("bv", [1, CL], f32, kind="ExternalInput").ap()
    kpT = nc.dram_tensor("kpT", [128, 2, PP], bf16, kind="ExternalInput").ap()
    vp = nc.dram_tensor("vp", [PP, HL, D + 1], bf16, kind="ExternalInput").ap()
    outT = nc.dram_tensor("outT", [E, S], f32, kind="ExternalOutput").ap()

    with tile.TileContext(nc) as tc:
        with tc.tile_pool(name="persist", bufs=1) as pers:
            # q-projection weights/bias first: they gate the very first
            # matmuls, so don't queue them behind the other ~1MB of DMAs
            wq_sb = pers.tile([128, 6, CL], bf16)
            nc.sync.dma_start(wq_sb[:], wqT.rearrange("(t p) c -> p t c", p=128))
            bq_sb = pers.tile([128, 2], f32)
            nc.sync.dma_start(bq_sb[:, 0:1], bq[0:128, :])
            nc.sync.dma_start(bq_sb[0:64, 1:2], bq[128:CL, :])

            wk_sb = pers.tile([128, 6, CL], bf16)
            wv_sb = pers.tile([128, 6, CL], bf16)
            wo_sb = pers.tile([128, 2, E], bf16)
            bk_sb = pers.tile([128, 2], f32)
            bvb_sb = pers.tile([128, CL], f32)
            kpT_sb = pers.tile([128, 2, PP], bf16)
            vp_sb = pers.tile([PP, HL, D + 1], bf16)

            # activations (all bf16)
            qT_sb = pers.tile([128, 2, S], bf16)
            kT_sb = pers.tile([128, 2, S], bf16)   # no prefix; kpT separate
            v_sb = pers.tile([128, NST, HL, D + 1], bf16)
            ctxT_sb = pers.tile([128, 2, S], bf16)
            expp_sb = pers.tile([PP, HL, S], bf16)  # prefix exp rows per head

            nc.vector.memset(v_sb[:, :, :, D:D + 1], 1.0)

            # ---------------- Phase 1a: Q / K projections ----------------
            # PE-bound prologue; ScalarE is idle here by design --
            # nothing downstream can run before qT/kT exist.
            with (
                tc.tile_pool(name="ps_proj", bufs=2, space="PSUM") as pp,
                tc.tile_pool(name="xq_pool", bufs=4) as xq_pool,
            ):
                def proj_block(xin, wsb, bsb, dst, sq, eng=None):
                    eng = eng or nc.sync
                    p0 = pp.tile([128, QT], f32, tag="p0", name="p0")
                    p1 = pp.tile([64, QT], f32, tag="p1", name="p1")
                    for ech in range(6):
                        xt = xq_pool.tile([128, QT], bf16, tag="xt",
                                          name="xt")
                        eng.dma_start(
                            xt[:],
                            xin[ech * 128:(ech + 1) * 128,
                                sq * QT:(sq + 1) * QT],
                        )
                        for n in range(QT // 512):
                            ns = slice(n * 512, (n + 1) * 512)
                            nc.tensor.matmul(
                                p0[:, ns], wsb[:, ech, 0:128], xt[:, ns],
                                start=(ech == 0), stop=(ech == 5),
                            )
                            nc.tensor.matmul(
                                p1[:, ns], wsb[:, ech, 128:CL], xt[:, ns],
                                start=(ech == 0), stop=(ech == 5),
                            )
                    ds = slice(sq * QT, (sq + 1) * QT)
                    nc.vector.tensor_scalar_add(
                        dst[:, 0, ds], p0[:], bsb[:, 0:1])
                    nc.vector.tensor_scalar_add(
                        dst[0:64, 1, ds], p1[:], bsb[0:64, 1:2])

                proj_block(xqT, wq_sb, bq_sb, qT_sb, 0)
                # now that the critical q DMAs are queued, stream in the
                # remaining weights behind them
                nc.sync.dma_start(
                    wk_sb[:], wkT.rearrange("(t p) c -> p t c", p=128))
                nc.sync.dma_start(bk_sb[:, 0:1], bk[0:128, :])
                nc.sync.dma_start(bk_sb[0:64, 1:2], bk[128:CL, :])
                nc.sync.dma_start(kpT_sb[:], kpT[:])
                nc.sync.dma_start(
                    wv_sb[:], wvT.rearrange("(t p) c -> p t c", p=128))
                nc.sync.dma_start(bvb_sb[:], bv.to_broadcast((128, CL)))
                nc.sync.dma_start(vp_sb[:], vp[:])
                nc.sync.dma_start(wo_sb[:, 0, :], woT[0:128, :])
                nc.sync.dma_start(wo_sb[0:64, 1, :], woT[128:CL, :])

                # sq0 prompt-prefix scores+exp only need qT(sq0)+kpT: emit
                # them before the K projections so ScalarE starts (and pays
                # the one-time exp table load) ~25us earlier.  psp borrows a
                # ps_proj "p1" slot, so no extra PSUM pressure.
                for h in range(HL):
                    pr, po = h // 2, 64 * (h % 2)
                    psp0 = pp.tile([PP, QT], f32, tag="p1", name="psp0")
                    for n in range(QT // 512):
                        ns = slice(n * 512, (n + 1) * 512)
                        nc.tensor.matmul(
                            psp0[:, ns],
                            kpT_sb[po:po + 64, pr, :],
                            qT_sb[po:po + 64, pr, ns],
                            start=True, stop=True,
                        )
                    nc.scalar.activation(
                        expp_sb[:, h, 0:QT], psp0[:],
                        EXP, scale=float(D) ** -0.5,
                    )

                for sq in range(NSQ):
                    proj_block(xkT, wk_sb, bk_sb, kT_sb, sq)

            # ---------- attention + V-proj + out-proj: one slot stream ----------
            with (
                tc.tile_pool(name="ps_s", bufs=2, space="PSUM") as ps_s,
                tc.tile_pool(name="ps_c", bufs=1, space="PSUM") as ps_c,
                tc.tile_pool(name="ps_sm", bufs=2, space="PSUM") as ps_sm,
                tc.tile_pool(name="expt_pool", bufs=20) as expt_pool,
                tc.tile_pool(name="nrm_pool", bufs=2) as nrm_pool,
                tc.tile_pool(name="xv_pool", bufs=8) as xv_pool,
                tc.tile_pool(name="xq2_pool", bufs=7) as xq2_pool,
                tc.tile_pool(name="out_pool", bufs=4) as out_pool,
            ):
                def emit_prefix(sq, h):
                    pr, po = h // 2, 64 * (h % 2)
                    psp = ps_s.tile([PP, QT], f32, tag="pss", name="psp")
                    for n in range(QT // 512):
                        ns = slice(n * 512, (n + 1) * 512)
                        qs = slice(sq * QT + n * 512, sq * QT + (n + 1) * 512)
                        nc.tensor.matmul(
                            psp[:, ns],
                            kpT_sb[po:po + 64, pr, :],
                            qT_sb[po:po + 64, pr, qs],
                            start=True, stop=True,
                        )
                    nc.scalar.activation(
                        expp_sb[:, h, sq * QT:(sq + 1) * QT], psp[:],
                        EXP, scale=float(D) ** -0.5,
                    )

                # Background q-projection for sq 1..3 (op-granular, drained
                # one op per stream slot using the time-multiplexed sm pool)
                def make_bg_qproj(sq):
                    ops = []
                    state = {}

                    def dma_op():
                        tiles = []
                        for ech in range(6):
                            xt2 = xq2_pool.tile([128, QT], bf16, tag="xt2",
                                                name="xt2")
                            nc.sync.dma_start(
                                xt2[:],
                                xqT[ech * 128:(ech + 1) * 128,
                                    sq * QT:(sq + 1) * QT],
                            )
                            tiles.append(xt2)
                        state["xt"] = tiles

                    ops.append(dma_op)

                    def mk_mm(c, grp, ech):
                        def op():
                            if ech == 0:
                                state[(c, grp)] = ps_sm.tile(
                                    [128, 512], f32, tag="sm", name="pq")
                            pt = state[(c, grp)]
                            rows = 128 if grp == 0 else 64
                            wc = slice(0, 128) if grp == 0 else slice(128, CL)
                            nc.tensor.matmul(
                                pt[0:rows, :], wq_sb[:, ech, wc],
                                state["xt"][ech][:, c * 512:(c + 1) * 512],
                                start=(ech == 0), stop=(ech == 5),
                            )
                        return op

                    def mk_evac(c, grp):
                        def op():
                            pt = state.pop((c, grp))
                            qs = slice(sq * QT + c * 512,
                                       sq * QT + (c + 1) * 512)
                            if grp == 0:
                                nc.vector.tensor_scalar_add(
                                    qT_sb[:, 0, qs], pt[:, :], bq_sb[:, 0:1])
                            else:
                                nc.vector.tensor_scalar_add(
                                    qT_sb[0:64, 1, qs], pt[0:64, :],
                                    bq_sb[0:64, 1:2])
                        return op

                    for c in range(QT // 512):
                        for grp in range(2):
                            for ech in range(6):
                                ops.append(mk_mm(c, grp, ech))
                            ops.append(mk_evac(c, grp))
                    for h in range(HL):
                        ops.append(lambda h=h: emit_prefix(sq, h))
                    return ops

                bg_work = []
                for nb, sqb in ((32, 1), (66, 2), (150, 3)):
                    for op in make_bg_qproj(sqb):
                        bg_work.append((nb, op))

                # xv DMA loads, one sq-group of 6 chunks at a time
                xvts = {}

                def load_xv(sqx):
                    tiles = []
                    for ech in range(6):
                        xvt = xv_pool.tile([128, QT], bf16, tag="xvt",
                                           name="xvt")
                        nc.sync.dma_start(
                            xvt[:],
                            xvT[ech * 128:(ech + 1) * 128,
                                sqx * QT:(sqx + 1) * QT],
                        )
                        tiles.append(xvt)
                    xvts[sqx] = tiles

                def emit_vproj(st):
                    sqx, stl = st // (QT // 128), st % (QT // 128)
                    if st == 0:
                        load_xv(0)
                    if stl == 0 and sqx + 1 < NSQ:
                        load_xv(sqx + 1)
                    pv = ps_sm.tile([128, 512], f32, tag="sm", name="pv")
                    for ech in range(6):
                        nc.tensor.matmul(
                            pv[:, 0:CL],
                            xvts[sqx][ech][:, stl * 128:(stl + 1) * 128],
                            wv_sb[:, ech, :],
                            start=(ech == 0), stop=(ech == 5),
                        )
                    nc.vector.tensor_add(
                        v_sb[:, st, :, 0:D],
                        pv[:, 0:CL].rearrange("p (h d) -> p h d", h=HL),
                        bvb_sb[:].rearrange("p (h d) -> p h d", h=HL),
                    )
                    if stl == (QT // 128) - 1:
                        del xvts[sqx]

                def emit_scores_exp(sq, h, kt):
                    pr, po = h // 2, 64 * (h % 2)
                    lhsT_k = kT_sb[po:po + 64, pr, kt * 128:(kt + 1) * 128]
                    pss = ps_s.tile([128, QT], f32, tag="pss", name="pss")
                    expt = expt_pool.tile([128, QT], bf16, tag="expt",
                                          name="expt")
                    for n in range(QT // 512):
                        ns = slice(n * 512, (n + 1) * 512)
                        qs = slice(sq * QT + n * 512, sq * QT + (n + 1) * 512)
                        nc.tensor.matmul(
                            pss[:, ns], lhsT_k, qT_sb[po:po + 64, pr, qs],
                            start=True, stop=True,
                        )
                    nc.scalar.activation(
                        expt[:], pss[:], EXP, scale=float(D) ** -0.5,
                    )
                    return expt

                psc_tiles = {}

                def emit_ctx(sq, h, kt, expt):
                    key = (sq, h)
                    if kt == 0:
                        psc_tiles[key] = ps_c.tile([D + 1, QT], f32,
                                                   tag="psc", name="psc")
                    psc = psc_tiles[key]
                    for n in range(QT // 512):
                        ns = slice(n * 512, (n + 1) * 512)
                        nc.tensor.matmul(
                            psc[:, ns], v_sb[:, kt, h, :], expt[:, ns],
                            start=(kt == 0), stop=(kt == NKT - 1),
                        )
                    if kt == TRAIL - 1:
                        # prompt-prefix ctx contribution (reads expp_sb rows)
                        for n in range(QT // 512):
                            ns = slice(n * 512, (n + 1) * 512)
                            qs = slice(sq * QT + n * 512,
                                       sq * QT + (n + 1) * 512)
                            nc.tensor.matmul(
                                psc[:, ns], vp_sb[:, h, :],
                                expp_sb[:, h, qs],
                                start=False, stop=False,
                            )
                    if kt == NKT - 1:
                        emit_norm(sq, h, psc)
                        del psc_tiles[key]

                def emit_norm(sq, h, psc):
                    pr, po = h // 2, 64 * (h % 2)
                    rc = nrm_pool.tile([1, QT], f32, tag="rc", name="rc")
                    nc.vector.reciprocal(rc[:], psc[D:D + 1, :])
                    rb = nrm_pool.tile([64, QT], f32, tag="rb", name="rb")
                    nc.gpsimd.partition_broadcast(rb[:], rc[:])
                    nc.vector.tensor_mul(
                        ctxT_sb[po:po + 64, pr, sq * QT:(sq + 1) * QT],
                        psc[0:D, :], rb[:],
                    )

                outproj_work = []

                def emit_outproj(sq):
                    # queue the 12 out-projection tiles; drained 1/slot so
                    # they never lump up in front of scores matmuls
                    for et in range(6):
                        for n in range(QT // 512):
                            outproj_work.append((et, sq * 2 + n))

                def emit_outproj_tile(et, qn):
                    es = slice(et * 128, (et + 1) * 128)
                    qs = slice(qn * 512, (qn + 1) * 512)
                    po3 = ps_sm.tile([128, 512], f32, tag="sm", name="po3")
                    nc.tensor.matmul(
                        po3[:], wo_sb[:, 0, es], ctxT_sb[:, 0, qs],
                        start=True, stop=False,
                    )
                    nc.tensor.matmul(
                        po3[:], wo_sb[0:64, 1, es], ctxT_sb[0:64, 1, qs],
                        start=False, stop=True,
                    )
                    ot = out_pool.tile([128, 512], f32, tag="ot", name="ot")
                    nc.vector.tensor_copy(ot[:], po3[:])
                    nc.sync.dma_start(outT[es, qs], ot[:])

                slots = [(sq, h, kt)
                         for sq in range(NSQ)
                         for h in range(HL)
                         for kt in range(NKT)]
                # ctx trails scores by TRAIL slots; a block-opening ctx
                # (kt==0, start=True) trails by TRAIL+GAP so the previous
                # block's norm chain (recip -> bcast -> mul, ~4us) can free
                # the single psc slot without stalling the PE queue.  The
                # stream catches back up popping 2 ctxs per slot.
                GAP = 6
                pending = []

                def pop_one():
                    (s2, e2) = pending.pop(0)
                    emit_ctx(*s2, e2)
                    if s2[2] == NKT - 1 and s2[1] == HL - 1:
                        emit_outproj(s2[0])

                vst = 0
                for j, slot in enumerate(slots):
                    # scores matmuls first in each slot so exp (the
                    # bottleneck engine's feed) is never queue-delayed
                    expt = emit_scores_exp(*slot)
                    pending.append((slot, expt))
                    if vst < NST:
                        emit_vproj(vst)
                        vst += 1
                    # near the stream end the trail no longer buys ScalarE
                    # slack -- drain it so the final norm/out-proj/store
                    # chain starts as early as possible
                    trail_eff = TRAIL if j < len(slots) - 34 else 2
                    for _ in range(3):
                        if not pending:
                            break
                        need = (trail_eff + GAP if pending[0][0][2] == 0
                                else trail_eff)
                        if len(pending) > need:
                            pop_one()
                        else:
                            break
                    if bg_work and j >= bg_work[0][0]:
                        bg_work.pop(0)[1]()
                    elif outproj_work:
                        emit_outproj_tile(*outproj_work.pop(0))
                while pending:
                    pop_one()
                    if outproj_work:
                        emit_outproj_tile(*outproj_work.pop(0))
                for _, op in bg_work:
                    op()
                while outproj_work:
                    emit_outproj_tile(*outproj_work.pop(0))

    nc.compile()
    return nc


def _get_nc():
    with _lock:
        if "nc" not in _compiled:
            _compiled["nc"] = _build()
        return _compiled["nc"]


def _prep_in_maps(query, key, value, prompt, Wq, bq, Wk, bk, Wv, bv, Wo, bo):
    f32 = np.float32
    qT = [np.ascontiguousarray(query[b].T).astype(BF16) for b in range(B)]
    kT = [np.ascontiguousarray(key[b].T).astype(BF16) for b in range(B)]
    vT = [np.ascontiguousarray(value[b].T).astype(BF16) for b in range(B)]
    in_maps = []
    for core in range(NCORES):
        b, g = core // NG, core % NG
        cs = slice(g * CL, (g + 1) * CL)
        kp = np.zeros((128, 2, PP), BF16)
        vpa = np.zeros((PP, HL, D + 1), BF16)
        vpa[:, :, D] = 1.0
        for h in range(HL):
            gh = g * HL + h
            kp[64 * (h % 2):64 * (h % 2) + 64, h // 2, :] = (
                prompt[b, 0, :, gh, :].T.astype(BF16))
            vpa[:, h, 0:D] = prompt[b, 1, :, gh, :].astype(BF16)
        in_maps.append({
            "xqT": qT[b], "xkT": kT[b], "xvT": vT[b],
            "wqT": np.ascontiguousarray(Wq[cs, :].T).astype(BF16),
            "wkT": np.ascontiguousarray(Wk[cs, :].T).astype(BF16),
            "wvT": np.ascontiguousarray(Wv[cs, :].T).astype(BF16),
            "woT": np.ascontiguousarray(Wo[:, cs].T).astype(BF16),
            "bq": np.ascontiguousarray(bq[cs]).astype(f32).reshape(CL, 1),
            "bk": np.ascontiguousarray(bk[cs]).astype(f32).reshape(CL, 1),
            "bv": np.ascontiguousarray(bv[cs]).astype(f32).reshape(1, CL),
            "kpT": kp, "vp": vpa,
        })
    return in_maps


def _combine(results, bo):
    out = np.empty((B, S, E), np.float32)
    for b in range(B):
        acc = results[b * NG]["outT"].astype(np.float32)
        for g in range(1, NG):
            acc = acc + results[b * NG + g]["outT"]
        out[b] = acc.T
    if bo is not None and np.any(bo):
        out += np.asarray(bo, np.float32)
    return out


def run(inputs, trace=False):
    """Returns (output, exec_time_ns or None)."""
    from concourse import bass_utils

    nc = _get_nc()
    in_maps = _prep_in_maps(**{k: np.asarray(v) for k, v in inputs.items()})
    bo = np.asarray(inputs["bo"])
    res = bass_utils.run_bass_kernel_spmd(
        nc, in_maps, core_ids=list(range(NCORES)), trace=trace,
    )
    return _combine(res.results, bo), res.exec_time_ns


def kernel(**inputs):
    out, _ = run(inputs)
    return out



# revision 16
# speedup vs baseline: 1.1484x; 1.1484x over previous
"""Multi-head attention (B=2,S=4096,E=768,H=12,D=64 + 16-token K/V prompt
prefix) on 8 Trainium2 NeuronCores.

Sharding: 2 batches x 4 head-groups (3 heads each). Each core computes QKV
projections for its 3 heads, full attention over its batch, and a partial
output projection (its 192 ctx channels); the host sums the 4 partials per
batch.

Key structure (v3):
- Uniform 33-k-tile pipeline: k-tile 32 holds the 16-token prompt prefix plus
  112 zero-padded keys whose v/ones-column are zero, so the pads contribute
  nothing to numerator or denominator.  No special-case prefix path.
- scoresT[k,q] psum from kT/qT slices (contraction D=64); exp is split across
  the Activation engine (exact, ~70% of tiles) and the Vector engine (~30%,
  Schraudolph int16/bf16-bitcast approximate exp: t = s*A+B -> int16 ->
  reinterpret as bf16 ~= e^s, max ~3.5% elementwise, ~1e-2 final L2).
  Triple-buffered scores psum decouples the PE->exp handshake.
- ctx computed in [q, d] orientation: lhsT = exp tile [128k, 128q], rhs =
  v_aug [128k, 65] (64 v dims + ones column) -> psum chunk [128q, 65]
  accumulated over the 33 k-tiles; the 65th column is the softmax
  denominator.  PSUM zero-region rule (one open accumulation group per 2KB
  bank): each [128, 4, 65] half-group lives in one bank with a single
  start (first matmul) / stop (last matmul).
- Normalized ctx [q, d] transposed back to [d, q] with PE-array transposes
  (identity matmul); transposes and out-projection tiles borrow the ctx psum
  bank pool between head groups.
- All Q/K/V projections run in a PE-only prologue (Act idles there but is
  not the bottleneck).
"""

import sys
import threading

import numpy as np

if "/opt/trn_rl_repo" not in sys.path:
    sys.path.insert(0, "/opt/trn_rl_repo")

import ml_dtypes

BF16 = ml_dtypes.bfloat16

B, S, E, H, D, PP = 2, 4096, 768, 12, 64, 16
NCORES = 8
NG = 4            # head-groups (tensor parallel)
HL = H // NG      # 3 local heads
CL = HL * D       # 192 local channels
NKT = 33          # 32 key tiles + 1 prefix/pad tile
SKVP = NKT * 128  # padded K/V length 4224
QT = 1024         # q block width
NSQ = S // QT     # 4
TRAIL = 6         # ctx matmuls trail scores by this many slots
GAP = 6           # extra trail after a head boundary (psum pool turnover)
APPROX_MOD = 2    # j%2 < APPROX_CNT -> DVE bit-trick exp (50%)
APPROX_CNT = 1

# Schraudolph constants for bf16 bitcast exp: t = int16(s*SCHA + SCHB),
# bitcast(t) ~= e^s.  SCHA folds the 1/sqrt(D) logit scale.  SCHB centers
# the linear-mantissa interpolation error (C=0.0436 -> +-3% elementwise).
SCHA = (128.0 / np.log(2.0)) * 0.125
SCHB = 127.0 * 128.0 - 128.0 * 0.0436

_lock = threading.Lock()
_compiled = {}


def _build():
    import concourse.bass as bass  # noqa: F401
    import concourse.mybir as mybir
    import concourse.tile as tile
    from concourse import bacc

    f32 = mybir.dt.float32
    bf16 = mybir.dt.bfloat16
    i16 = mybir.dt.int16
    EXP = mybir.ActivationFunctionType.Exp
    MUL = mybir.AluOpType.mult
    ADD = mybir.AluOpType.add

    nc = bacc.Bacc("TRN2", target_bir_lowering=False, debug=False)

    xqT = nc.dram_tensor("xqT", [E, S], bf16, kind="ExternalInput").ap()
    xkT = nc.dram_tensor("xkT", [E, S], bf16, kind="ExternalInput").ap()
    xvT = nc.dram_tensor("xvT", [E, S], bf16, kind="ExternalInput").ap()
    wqT = nc.dram_tensor("wqT", [E, CL], bf16, kind="ExternalInput").ap()
    wkT = nc.dram_tensor("wkT", [E, CL], bf16, kind="ExternalInput").ap()
    wvT = nc.dram_tensor("wvT", [E, CL], bf16, kind="ExternalInput").ap()
    woT = nc.dram_tensor("woT", [CL, E], bf16, kind="ExternalInput").ap()
    bq = nc.dram_tensor("bq", [CL, 1], f32, kind="ExternalInput").ap()
    bk = nc.dram_tensor("bk", [CL, 1], f32, kind="ExternalInput").ap()
    bv = nc.dram_tensor("bv", [1, CL], f32, kind="ExternalInput").ap()
    kpre = nc.dram_tensor("kpre", [128, 2, 128], bf16, kind="ExternalInput").ap()
    vpre = nc.dram_tensor("vpre", [128, HL, D + 1], bf16,
                          kind="ExternalInput").ap()
    ident = nc.dram_tensor("ident", [128, 128], bf16, kind="ExternalInput").ap()
    outT = nc.dram_tensor("outT", [E, S], f32, kind="ExternalOutput").ap()

    with tile.TileContext(nc) as tc:
        with tc.tile_pool(name="persist", bufs=1) as pers:
            # q-projection weights/bias first: they gate the first matmuls
            wq_sb = pers.tile([128, 6, CL], bf16)
            nc.sync.dma_start(wq_sb[:], wqT.rearrange("(t p) c -> p t c", p=128))
            bq_sb = pers.tile([128, 2], f32)
            nc.sync.dma_start(bq_sb[:, 0:1], bq[0:128, :])
            nc.sync.dma_start(bq_sb[0:64, 1:2], bq[128:CL, :])

            wk_sb = pers.tile([128, 6, CL], bf16)
            wv_sb = pers.tile([128, 6, CL], bf16)
            wo_sb = pers.tile([128, 2, E], bf16)
            bk_sb = pers.tile([128, 2], f32)
            bvb_sb = pers.tile([128, CL], f32)
            id_sb = pers.tile([128, 128], bf16)

            # activations (bf16)
            qT_sb = pers.tile([128, 2, S], bf16)
            kT_sb = pers.tile([128, 2, SKVP], bf16)
            v_sb = pers.tile([128, NKT, HL, D + 1], bf16)
            ctxT_sb = pers.tile([128, 2, S], bf16)

            with (
                tc.tile_pool(name="ps_s", bufs=3, space="PSUM") as ps_s,
                tc.tile_pool(name="ps_c", bufs=2, space="PSUM") as ps_c,
                tc.tile_pool(name="x_pool", bufs=5) as x_pool,
                tc.tile_pool(name="expt_pool", bufs=TRAIL + GAP + 6) as expt_pool,
                tc.tile_pool(name="ctxb_pool", bufs=2) as ctxb_pool,
                tc.tile_pool(name="nrm_pool", bufs=2) as nrm_pool,
                tc.tile_pool(name="out_pool", bufs=4) as out_pool,
            ):
                # ------------- prologue: all Q/K/V projections --------------
                def load_x(src, blk, split=False):
                    xt = x_pool.tile([128, 6, QT], bf16, tag="x", name="x")
                    view = src.rearrange("(t p) s -> p t s", p=128)[
                        :, :, blk * QT:(blk + 1) * QT]
                    if split:
                        # one DMA per contraction chunk so the first matmul
                        # starts as soon as chunk 0 lands
                        for ech in range(6):
                            nc.sync.dma_start(xt[:, ech, :], view[:, ech, :])
                    else:
                        nc.sync.dma_start(xt[:], view)
                    return xt

                def proj_block(xt, wsb, bsb, dst, blk):
                    p0 = ps_s.tile([128, QT], f32, tag="s", name="p0")
                    p1t = ps_s.tile([128, QT], f32, tag="s", name="p1t")
                    for ech in range(6):
                        for n in range(QT // 512):
                            ns = slice(n * 512, (n + 1) * 512)
                            nc.tensor.matmul(
                                p0[:, ns], wsb[:, ech, 0:128], xt[:, ech, ns],
                                start=(ech == 0), stop=(ech == 5),
                            )
                            nc.tensor.matmul(
                                p1t[0:64, ns], wsb[:, ech, 128:CL],
                                xt[:, ech, ns],
                                start=(ech == 0), stop=(ech == 5),
                            )
                    ds = slice(blk * QT, (blk + 1) * QT)
                    nc.vector.tensor_scalar_add(
                        dst[:, 0, ds], p0[:], bsb[:, 0:1])
                    nc.vector.tensor_scalar_add(
                        dst[0:64, 1, ds], p1t[0:64, :], bsb[0:64, 1:2])

                def emit_vproj(xt, st):
                    stl = st % 8
                    pvt = ps_s.tile([128, QT], f32, tag="s", name="pvt")
                    pv = pvt[:, 0:CL]
                    for ech in range(6):
                        nc.tensor.matmul(
                            pv,
                            xt[:, ech, stl * 128:(stl + 1) * 128],
                            wv_sb[:, ech, :],
                            start=(ech == 0), stop=(ech == 5),
                        )
                    nc.vector.tensor_add(
                        v_sb[:, st, :, 0:D],
                        pv.rearrange("p (h d) -> p h d", h=HL),
                        bvb_sb[:].rearrange("p (h d) -> p h d", h=HL),
                    )

                xq0 = load_x(xqT, 0, split=True)
                # critical-path DMAs for the K projection next
                nc.sync.dma_start(
                    wk_sb[:], wkT.rearrange("(t p) c -> p t c", p=128))
                nc.sync.dma_start(bk_sb[:, 0:1], bk[0:128, :])
                nc.sync.dma_start(bk_sb[0:64, 1:2], bk[128:CL, :])
                xk0 = load_x(xkT, 0)
                nc.sync.dma_start(
                    wv_sb[:], wvT.rearrange("(t p) c -> p t c", p=128))
                nc.sync.dma_start(bvb_sb[:], bv.to_broadcast((128, CL)))
                xv0 = load_x(xvT, 0)
                proj_block(xq0, wq_sb, bq_sb, qT_sb, 0)

                # remaining non-critical inputs
                nc.sync.dma_start(kT_sb[:, :, S:SKVP], kpre[:])
                nc.sync.dma_start(v_sb[:, NKT - 1, :, :], vpre[:])
                nc.sync.dma_start(wo_sb[:, 0, :], woT[0:128, :])
                nc.sync.dma_start(wo_sb[0:64, 1, :], woT[128:CL, :])
                nc.sync.dma_start(id_sb[:], ident[:])
                nc.vector.memset(v_sb[:, 0:NKT - 1, :, D:D + 1], 1.0)

                # serial pre-stream work: K block 0 (gates the first scores)
                # and V tiles 0..7 (gate the first ctx pops)
                proj_block(xk0, wk_sb, bk_sb, kT_sb, 0)
                for st in range(8):
                    emit_vproj(xv0, st)

                # The remaining projections (K1-3, V8-31, Q1-3) drain into the
                # early slot stream as small units (~2 per slot) so the exp
                # engines start ~50us earlier.  x-tile loads are embedded with
                # enough lead; deadlines: K block b by slot 8b, V tile st by
                # slot st+TRAIL, Q block b by slot 99b.
                pro = []
                xts = {}

                def pro_load(src, key):
                    def u():
                        xts[key] = load_x(src, key[1])
                    return u

                def pro_proj_mm(key, wsb, dst_half, n):
                    def u():
                        if n == 0:
                            xts[(key, "p", dst_half)] = ps_s.tile(
                                [128, QT], f32, tag="s", name="pp")
                        pt = xts[(key, "p", dst_half)]
                        rows = 128 if dst_half == 0 else 64
                        wc = slice(0, 128) if dst_half == 0 else slice(128, CL)
                        ns = slice(n * 512, (n + 1) * 512)
                        for ech in range(6):
                            nc.tensor.matmul(
                                pt[0:rows, ns], wsb[:, ech, wc],
                                xts[key][:, ech, ns],
                                start=(ech == 0), stop=(ech == 5),
                            )
                    return u

                def pro_proj_evac(key, bsb, dst, blk, dst_half):
                    def u():
                        pt = xts.pop((key, "p", dst_half))
                        ds = slice(blk * QT, (blk + 1) * QT)
                        if dst_half == 0:
                            nc.vector.tensor_scalar_add(
                                dst[:, 0, ds], pt[:], bsb[:, 0:1])
                        else:
                            nc.vector.tensor_scalar_add(
                                dst[0:64, 1, ds], pt[0:64, :],
                                bsb[0:64, 1:2])
                    return u

                def pro_vproj(key, st):
                    def u():
                        emit_vproj(xts[key], st)
                    return u

                def proj_units(key, wsb, bsb, dst, blk):
                    return [
                        pro_proj_mm(key, wsb, 0, 0), pro_proj_mm(key, wsb, 0, 1),
                        pro_proj_evac(key, bsb, dst, blk, 0),
                        pro_proj_mm(key, wsb, 1, 0), pro_proj_mm(key, wsb, 1, 1),
                        pro_proj_evac(key, bsb, dst, blk, 1),
                    ]

                # prefetch the first stream-phase loads
                xts[("k", 1)] = load_x(xkT, 1)
                xts[("k", 2)] = load_x(xkT, 2)
                xts[("v", 1)] = load_x(xvT, 1)
                xts[("v", 0)] = xv0

                pro += proj_units(("k", 1), wk_sb, bk_sb, kT_sb, 1)
                pro += [pro_vproj(("v", 1), st) for st in range(8, 12)]
                pro.append(pro_load(xvT, ("v", 2)))
                pro += proj_units(("k", 2), wk_sb, bk_sb, kT_sb, 2)
                pro += [pro_vproj(("v", 1), st) for st in range(12, 16)]
                pro.append(pro_load(xkT, ("k", 3)))
                pro += [pro_vproj(("v", 2), st) for st in range(16, 20)]
                pro += proj_units(("k", 3), wk_sb, bk_sb, kT_sb, 3)
                pro.append(pro_load(xqT, ("q", 1)))
                pro += [pro_vproj(("v", 2), st) for st in range(20, 24)]
                pro.append(pro_load(xvT, ("v", 3)))
                pro += proj_units(("q", 1), wq_sb, bq_sb, qT_sb, 1)
                pro += [pro_vproj(("v", 3), st) for st in range(24, 28)]
                pro.append(pro_load(xqT, ("q", 2)))
                pro += proj_units(("q", 2), wq_sb, bq_sb, qT_sb, 2)
                pro += [pro_vproj(("v", 3), st) for st in range(28, 32)]
                pro.append(pro_load(xqT, ("q", 3)))
                pro += proj_units(("q", 3), wq_sb, bq_sb, qT_sb, 3)

                # ---------------- slot stream -------------------------------
                bg = []

                def bg_transpose(ctxb, j, po, pr, qs):
                    def u():
                        tp = ps_c.tile([128, 512], f32, tag="big", name="tp")
                        tpv = tp[0:64, 0:64].bitcast(bf16)
                        nc.tensor.transpose(tpv, ctxb[:, j, :], id_sb[:])
                        nc.scalar.copy(ctxT_sb[po:po + 64, pr, qs], tpv)
                    return u

                def bg_outproj(et, qn):
                    def u():
                        es = slice(et * 128, (et + 1) * 128)
                        qs = slice(qn * 512, (qn + 1) * 512)
                        po3 = ps_c.tile([128, 512], f32, tag="big", name="po3")
                        nc.tensor.matmul(
                            po3[:], wo_sb[:, 0, es], ctxT_sb[:, 0, qs],
                            start=True, stop=False,
                        )
                        nc.tensor.matmul(
                            po3[:], wo_sb[0:64, 1, es], ctxT_sb[0:64, 1, qs],
                            start=False, stop=True,
                        )
                        ot = out_pool.tile([128, 512], f32, tag="ot", name="ot")
                        nc.scalar.copy(ot[:], po3[:])
                        nc.sync.dma_start(outT[es, qs], ot[:])
                    return u

                def emit_scores_exp(j, blk, h, kt):
                    pr, po = h // 2, 64 * (h % 2)
                    lhsT_k = kT_sb[po:po + 64, pr, kt * 128:(kt + 1) * 128]
                    pss = ps_s.tile([128, QT], f32, tag="s", name="pss")
                    expt = expt_pool.tile([128, QT], bf16, tag="expt",
                                          name="expt")
                    for n in range(QT // 512):
                        ns = slice(n * 512, (n + 1) * 512)
                        qs = slice(blk * QT + n * 512, blk * QT + (n + 1) * 512)
                        nc.tensor.matmul(
                            pss[:, ns], lhsT_k, qT_sb[po:po + 64, pr, qs],
                            start=True, stop=True,
                        )
                    if j % APPROX_MOD < APPROX_CNT:
                        nc.vector.tensor_scalar(
                            expt[:].bitcast(i16), pss[:],
                            float(SCHA), float(SCHB), MUL, ADD,
                        )
                    else:
                        nc.scalar.activation(
                            expt[:], pss[:], EXP, scale=0.125,
                        )
                    return expt

                state = {}

                def emit_ctx(blk, h, kt, expt):
                    if kt == 0:
                        ta = ps_c.tile([128, 512], f32, tag="big", name="psca")
                        tb = ps_c.tile([128, 512], f32, tag="big", name="pscb")
                        state["psc"] = (
                            ta[:, 0:4 * (D + 1)].rearrange(
                                "p (a b) -> p a b", a=4),
                            tb[:, 0:4 * (D + 1)].rearrange(
                                "p (a b) -> p a b", a=4),
                        )
                    psca, pscb = state["psc"]
                    first, last = (kt == 0), (kt == NKT - 1)
                    for jj in range(8):
                        pv = psca if jj < 4 else pscb
                        cc = jj % 4
                        # one accumulation group per psum bank: only the
                        # bank's first matmul starts (lazy-zeroing the whole
                        # bank), only its final matmul stops
                        nc.tensor.matmul(
                            pv[:, cc, :], expt[:, jj * 128:(jj + 1) * 128],
                            v_sb[:, kt, h, :],
                            start=(first and cc == 0), stop=(last and cc == 3),
                        )
                    if last:
                        emit_norm(blk, h, psca, pscb)

                def emit_norm(blk, h, psca, pscb):
                    pr, po = h // 2, 64 * (h % 2)
                    rc = nrm_pool.tile([128, 8], f32, tag="rc", name="rc")
                    nc.vector.reciprocal(rc[:, 0:4], psca[:, :, D])
                    nc.vector.reciprocal(rc[:, 4:8], pscb[:, :, D])
                    ctxb = ctxb_pool.tile([128, 8, D], bf16, tag="ctxb",
                                          name="ctxb")
                    for j in range(8):
                        pv = psca if j < 4 else pscb
                        nc.vector.tensor_scalar_mul(
                            ctxb[:, j, :], pv[:, j % 4, 0:D], rc[:, j:j + 1])
                    for j in range(8):
                        qs = slice(blk * QT + j * 128, blk * QT + (j + 1) * 128)
                        bg.append(bg_transpose(ctxb, j, po, pr, qs))
                    if h == HL - 1:
                        for et in range(6):
                            for n in range(QT // 512):
                                bg.append(bg_outproj(et, blk * 2 + n))

                slots = [(blk, h, kt)
                         for blk in range(NSQ)
                         for h in range(HL)
                         for kt in range(NKT)]
                pending = []

                def pop_one():
                    (s2, e2) = pending.pop(0)
                    emit_ctx(*s2, e2)

                for j, slot in enumerate(slots):
                    expt = emit_scores_exp(j, *slot)
                    pending.append((slot, expt))
                    trail_eff = TRAIL if j < len(slots) - 12 else 2
                    for _ in range(3):
                        if not pending:
                            break
                        nblk, nh, nkt = pending[0][0]
                        # head boundaries leave GAP slots (block boundaries a
                        # bit more) so the psum pool turns over (norm +
                        # transposes + outproj tiles) before the next group
                        need = trail_eff
                        if nkt == 0:
                            need += GAP + (2 if nh == 0 else 0)
                        if len(pending) > need:
                            pop_one()
                        else:
                            break
                    for _ in range(2):
                        if pro:
                            pro.pop(0)()
                    drain = 3 if len(bg) > 10 else 2
                    for _ in range(drain):
                        if bg:
                            bg.pop(0)()
                while pending:
                    pop_one()
                    if bg:
                        bg.pop(0)()
                while bg:
                    bg.pop(0)()
                assert not pro

    nc.compile()
    return nc


def _get_nc():
    with _lock:
        if "nc" not in _compiled:
            _compiled["nc"] = _build()
        return _compiled["nc"]


def _prep_in_maps(query, key, value, prompt, Wq, bq, Wk, bk, Wv, bv, Wo, bo):
    f32 = np.float32
    qT = [np.ascontiguousarray(query[b].T).astype(BF16) for b in range(B)]
    kT = [np.ascontiguousarray(key[b].T).astype(BF16) for b in range(B)]
    vT = [np.ascontiguousarray(value[b].T).astype(BF16) for b in range(B)]
    ident = np.eye(128, dtype=BF16)
    in_maps = []
    for core in range(NCORES):
        b, g = core // NG, core % NG
        cs = slice(g * CL, (g + 1) * CL)
        kp = np.zeros((128, 2, 128), BF16)
        vp = np.zeros((128, HL, D + 1), BF16)
        vp[0:PP, :, D] = 1.0
        for h in range(HL):
            gh = g * HL + h
            kp[64 * (h % 2):64 * (h % 2) + 64, h // 2, 0:PP] = (
                prompt[b, 0, :, gh, :].T.astype(BF16))
            vp[0:PP, h, 0:D] = prompt[b, 1, :, gh, :].astype(BF16)
        in_maps.append({
            "xqT": qT[b], "xkT": kT[b], "xvT": vT[b],
            "wqT": np.ascontiguousarray(Wq[cs, :].T).astype(BF16),
            "wkT": np.ascontiguousarray(Wk[cs, :].T).astype(BF16),
            "wvT": np.ascontiguousarray(Wv[cs, :].T).astype(BF16),
            "woT": np.ascontiguousarray(Wo[:, cs].T).astype(BF16),
            "bq": np.ascontiguousarray(bq[cs]).astype(f32).reshape(CL, 1),
            "bk": np.ascontiguousarray(bk[cs]).astype(f32).reshape(CL, 1),
            "bv": np.ascontiguousarray(bv[cs]).astype(f32).reshape(1, CL),
            "kpre": kp, "vpre": vp, "ident": ident,
        })
    return in_maps


def _combine(results, bo):
    out = np.empty((B, S, E), np.float32)
    for b in range(B):
        acc = results[b * NG]["outT"].astype(np.float32)
        for g in range(1, NG):
            acc = acc + results[b * NG + g]["outT"]
        out[b] = acc.T
    if bo is not None and np.any(bo):
        out += np.asarray(bo, np.float32)
    return out


def run(inputs, trace=False):
    """Returns (output, exec_time_ns or None)."""
    from concourse import bass_utils

    nc = _get_nc()
    in_maps = _prep_in_maps(**{k: np.asarray(v) for k, v in inputs.items()})
    bo = np.asarray(inputs["bo"])
    res = bass_utils.run_bass_kernel_spmd(
        nc, in_maps, core_ids=list(range(NCORES)), trace=trace,
    )
    return _combine(res.results, bo), res.exec_time_ns


def kernel(**inputs):
    out, _ = run(inputs)
    return out


# revision 21
# speedup vs baseline: 1.2001x; 1.0450x over previous
"""Multi-head attention (B=2,S=4096,E=768,H=12,D=64 + 16-token K/V prompt
prefix) on 8 Trainium2 NeuronCores.

Sharding: 2 batches x 4 head-groups (3 heads each). Each core computes QKV
projections for its 3 heads, full attention over its batch, and a partial
output projection (its 192 ctx channels); the host sums the 4 partials per
batch.

Key structure (v3):
- Uniform 33-k-tile pipeline: k-tile 32 holds the 16-token prompt prefix plus
  112 zero-padded keys whose v/ones-column are zero, so the pads contribute
  nothing to numerator or denominator.  No special-case prefix path.
- scoresT[k,q] psum from kT/qT slices (contraction D=64); exp is split across
  the Activation engine (exact, ~70% of tiles) and the Vector engine (~30%,
  Schraudolph int16/bf16-bitcast approximate exp: t = s*A+B -> int16 ->
  reinterpret as bf16 ~= e^s, max ~3.5% elementwise, ~1e-2 final L2).
  Triple-buffered scores psum decouples the PE->exp handshake.
- ctx computed in [q, d] orientation: lhsT = exp tile [128k, 128q], rhs =
  v_aug [128k, 65] (64 v dims + ones column) -> psum chunk [128q, 65]
  accumulated over the 33 k-tiles; the 65th column is the softmax
  denominator.  PSUM zero-region rule (one open accumulation group per 2KB
  bank): each [128, 4, 65] half-group lives in one bank with a single
  start (first matmul) / stop (last matmul).
- Normalized ctx [q, d] transposed back to [d, q] with PE-array transposes
  (identity matmul); transposes and out-projection tiles borrow the ctx psum
  bank pool between head groups.
- All Q/K/V projections run in a PE-only prologue (Act idles there but is
  not the bottleneck).
"""

import sys
import threading

import numpy as np

if "/opt/trn_rl_repo" not in sys.path:
    sys.path.insert(0, "/opt/trn_rl_repo")

import ml_dtypes

BF16 = ml_dtypes.bfloat16

B, S, E, H, D, PP = 2, 4096, 768, 12, 64, 16
NCORES = 8
NG = 4            # head-groups (tensor parallel)
HL = H // NG      # 3 local heads
CL = HL * D       # 192 local channels
NKT = 33          # 32 key tiles + 1 prefix/pad tile
SKVP = NKT * 128  # padded K/V length 4224
QT = 1024         # q block width
NSQ = S // QT     # 4
TRAIL = 6         # ctx matmuls trail scores by this many slots
# extra trail after each boundary so the shared psum pool turns over
# (before h1: norm only; before h2: norm + paired transposes; before a new
# block: norm + transposes + out-projection tiles)
GAPS = {0: 11, 1: 4, 2: 7}

# Schraudolph constants for bf16 bitcast exp: t = int16(s*SCHA + SCHB),
# bitcast(t) ~= e^s.  SCHA folds the 1/sqrt(D) logit scale.  SCHB centers
# the linear-mantissa interpolation error (C=0.0436 -> +-3% elementwise).
SCHA = (128.0 / np.log(2.0)) * 0.125
SCHB = 127.0 * 128.0 - 128.0 * 0.0436

_lock = threading.Lock()
_compiled = {}


def _build():
    import concourse.bass as bass  # noqa: F401
    import concourse.mybir as mybir
    import concourse.tile as tile
    from concourse import bacc

    f32 = mybir.dt.float32
    bf16 = mybir.dt.bfloat16
    i16 = mybir.dt.int16
    EXP = mybir.ActivationFunctionType.Exp
    MUL = mybir.AluOpType.mult
    ADD = mybir.AluOpType.add

    nc = bacc.Bacc("TRN2", target_bir_lowering=False, debug=False)

    xqT = nc.dram_tensor("xqT", [E, S], bf16, kind="ExternalInput").ap()
    xkT = nc.dram_tensor("xkT", [E, S], bf16, kind="ExternalInput").ap()
    xvT = nc.dram_tensor("xvT", [E, S], bf16, kind="ExternalInput").ap()
    wqT = nc.dram_tensor("wqT", [E, CL], bf16, kind="ExternalInput").ap()
    wkT = nc.dram_tensor("wkT", [E, CL], bf16, kind="ExternalInput").ap()
    wvT = nc.dram_tensor("wvT", [E, CL], bf16, kind="ExternalInput").ap()
    woT = nc.dram_tensor("woT", [CL, E], bf16, kind="ExternalInput").ap()
    bq = nc.dram_tensor("bq", [CL, 1], f32, kind="ExternalInput").ap()
    bk = nc.dram_tensor("bk", [CL, 1], f32, kind="ExternalInput").ap()
    bv = nc.dram_tensor("bv", [1, CL], f32, kind="ExternalInput").ap()
    kpre = nc.dram_tensor("kpre", [128, 2, 128], bf16, kind="ExternalInput").ap()
    vpre = nc.dram_tensor("vpre", [128, HL, D + 1], bf16,
                          kind="ExternalInput").ap()
    ident = nc.dram_tensor("ident", [128, 128], bf16, kind="ExternalInput").ap()
    outT = nc.dram_tensor("outT", [E, S], f32, kind="ExternalOutput").ap()

    with tile.TileContext(nc) as tc:
        with tc.tile_pool(name="persist", bufs=1) as pers:
            # q-projection weights/bias first: they gate the first matmuls
            wq_sb = pers.tile([128, 6, CL], bf16)
            nc.sync.dma_start(wq_sb[:], wqT.rearrange("(t p) c -> p t c", p=128))
            bq_sb = pers.tile([128, 2], f32)
            nc.sync.dma_start(bq_sb[:, 0:1], bq[0:128, :])
            nc.sync.dma_start(bq_sb[0:64, 1:2], bq[128:CL, :])

            wk_sb = pers.tile([128, 6, CL], bf16)
            wv_sb = pers.tile([128, 6, CL], bf16)
            wo_sb = pers.tile([128, 2, E], bf16)
            bk_sb = pers.tile([128, 2], f32)
            bvb_sb = pers.tile([128, CL], f32)
            id_sb = pers.tile([128, 128], bf16)

            # activations (bf16)
            qT_sb = pers.tile([128, 2, S], bf16)
            kT_sb = pers.tile([128, 2, SKVP], bf16)
            v_sb = pers.tile([128, NKT, HL, D + 1], bf16)
            ctxT_sb = pers.tile([128, 2, S], bf16)

            with (
                tc.tile_pool(name="ps_s", bufs=3, space="PSUM") as ps_s,
                tc.tile_pool(name="ps_c", bufs=2, space="PSUM") as ps_c,
                tc.tile_pool(name="x_pool", bufs=4) as x_pool,
                tc.tile_pool(name="expt_pool", bufs=TRAIL + 16) as expt_pool,
                tc.tile_pool(name="ctxb_pool", bufs=2) as ctxb_pool,
                tc.tile_pool(name="nrm_pool", bufs=2) as nrm_pool,
                tc.tile_pool(name="out_pool", bufs=6) as out_pool,
            ):
                # ------------- prologue: all Q/K/V projections --------------
                def load_x(src, blk, split=False):
                    xt = x_pool.tile([128, 6, QT], bf16, tag="x", name="x")
                    view = src.rearrange("(t p) s -> p t s", p=128)[
                        :, :, blk * QT:(blk + 1) * QT]
                    if split:
                        # one DMA per contraction chunk so the first matmul
                        # starts as soon as chunk 0 lands
                        for ech in range(6):
                            nc.sync.dma_start(xt[:, ech, :], view[:, ech, :])
                    else:
                        nc.sync.dma_start(xt[:], view)
                    return xt

                def proj_block(xt, wsb, bsb, dst, blk):
                    p0 = ps_s.tile([128, QT], f32, tag="s", name="p0")
                    p1t = ps_s.tile([128, QT], f32, tag="s", name="p1t")
                    for ech in range(6):
                        for n in range(QT // 512):
                            ns = slice(n * 512, (n + 1) * 512)
                            nc.tensor.matmul(
                                p0[:, ns], wsb[:, ech, 0:128], xt[:, ech, ns],
                                start=(ech == 0), stop=(ech == 5),
                            )
                            nc.tensor.matmul(
                                p1t[0:64, ns], wsb[:, ech, 128:CL],
                                xt[:, ech, ns],
                                start=(ech == 0), stop=(ech == 5),
                            )
                    ds = slice(blk * QT, (blk + 1) * QT)
                    nc.vector.tensor_scalar_add(
                        dst[:, 0, ds], p0[:], bsb[:, 0:1])
                    nc.vector.tensor_scalar_add(
                        dst[0:64, 1, ds], p1t[0:64, :], bsb[0:64, 1:2])

                def emit_vproj(xt, st):
                    stl = st % 8
                    pvt = ps_s.tile([128, QT], f32, tag="s", name="pvt")
                    pv = pvt[:, 0:CL]
                    for ech in range(6):
                        nc.tensor.matmul(
                            pv,
                            xt[:, ech, stl * 128:(stl + 1) * 128],
                            wv_sb[:, ech, :],
                            start=(ech == 0), stop=(ech == 5),
                        )
                    nc.vector.tensor_add(
                        v_sb[:, st, :, 0:D],
                        pv.rearrange("p (h d) -> p h d", h=HL),
                        bvb_sb[:].rearrange("p (h d) -> p h d", h=HL),
                    )

                xq0 = load_x(xqT, 0, split=True)
                # critical-path DMAs for the K projection next
                nc.sync.dma_start(
                    wk_sb[:], wkT.rearrange("(t p) c -> p t c", p=128))
                nc.sync.dma_start(bk_sb[:, 0:1], bk[0:128, :])
                nc.sync.dma_start(bk_sb[0:64, 1:2], bk[128:CL, :])
                xk0 = load_x(xkT, 0)
                nc.sync.dma_start(
                    wv_sb[:], wvT.rearrange("(t p) c -> p t c", p=128))
                nc.sync.dma_start(bvb_sb[:], bv.to_broadcast((128, CL)))
                xv0 = load_x(xvT, 0)
                proj_block(xq0, wq_sb, bq_sb, qT_sb, 0)

                # remaining non-critical inputs
                nc.sync.dma_start(kT_sb[:, :, S:SKVP], kpre[:])
                nc.sync.dma_start(v_sb[:, NKT - 1, :, :], vpre[:])
                nc.sync.dma_start(wo_sb[:, 0, :], woT[0:128, :])
                nc.sync.dma_start(wo_sb[0:64, 1, :], woT[128:CL, :])
                nc.sync.dma_start(id_sb[:], ident[:])
                nc.vector.memset(v_sb[:, 0:NKT - 1, :, D:D + 1], 1.0)

                # serial pre-stream work: K block 0 (gates the first scores)
                # and V tiles 0..7 (gate the first ctx pops)
                proj_block(xk0, wk_sb, bk_sb, kT_sb, 0)
                for st in range(8):
                    emit_vproj(xv0, st)

                # The remaining projections (K1-3, V8-31, Q1-3) drain into the
                # early slot stream as small units (~2 per slot) so the exp
                # engines start ~50us earlier.  x-tile loads are embedded with
                # enough lead; deadlines: K block b by slot 8b, V tile st by
                # slot st+TRAIL, Q block b by slot 99b.
                pro = []
                xts = {}

                def pro_load(src, key):
                    def u():
                        xts[key] = load_x(src, key[1])
                    return u

                def pro_proj_mm(key, wsb, dst_half, n):
                    def u():
                        if n == 0:
                            xts[(key, "p", dst_half)] = ps_s.tile(
                                [128, QT], f32, tag="s", name="pp")
                        pt = xts[(key, "p", dst_half)]
                        rows = 128 if dst_half == 0 else 64
                        wc = slice(0, 128) if dst_half == 0 else slice(128, CL)
                        ns = slice(n * 512, (n + 1) * 512)
                        for ech in range(6):
                            nc.tensor.matmul(
                                pt[0:rows, ns], wsb[:, ech, wc],
                                xts[key][:, ech, ns],
                                start=(ech == 0), stop=(ech == 5),
                            )
                    return u

                def pro_proj_evac(key, bsb, dst, blk, dst_half):
                    def u():
                        pt = xts.pop((key, "p", dst_half))
                        ds = slice(blk * QT, (blk + 1) * QT)
                        if dst_half == 0:
                            nc.vector.tensor_scalar_add(
                                dst[:, 0, ds], pt[:], bsb[:, 0:1])
                        else:
                            nc.vector.tensor_scalar_add(
                                dst[0:64, 1, ds], pt[0:64, :],
                                bsb[0:64, 1:2])
                    return u

                def pro_vproj(key, st):
                    def u():
                        emit_vproj(xts[key], st)
                    return u

                def proj_units(key, wsb, bsb, dst, blk):
                    return [
                        pro_proj_mm(key, wsb, 0, 0), pro_proj_mm(key, wsb, 0, 1),
                        pro_proj_evac(key, bsb, dst, blk, 0),
                        pro_proj_mm(key, wsb, 1, 0), pro_proj_mm(key, wsb, 1, 1),
                        pro_proj_evac(key, bsb, dst, blk, 1),
                    ]

                # prefetch the first stream-phase loads
                xts[("k", 1)] = load_x(xkT, 1)
                xts[("k", 2)] = load_x(xkT, 2)
                xts[("v", 1)] = load_x(xvT, 1)
                xts[("v", 0)] = xv0

                pro += proj_units(("k", 1), wk_sb, bk_sb, kT_sb, 1)
                pro += [pro_vproj(("v", 1), st) for st in range(8, 12)]
                pro.append(pro_load(xvT, ("v", 2)))
                pro += proj_units(("k", 2), wk_sb, bk_sb, kT_sb, 2)
                pro += [pro_vproj(("v", 1), st) for st in range(12, 16)]
                pro.append(pro_load(xkT, ("k", 3)))
                pro += [pro_vproj(("v", 2), st) for st in range(16, 20)]
                pro += proj_units(("k", 3), wk_sb, bk_sb, kT_sb, 3)
                pro.append(pro_load(xqT, ("q", 1)))
                pro += [pro_vproj(("v", 2), st) for st in range(20, 24)]
                pro.append(pro_load(xvT, ("v", 3)))
                pro += proj_units(("q", 1), wq_sb, bq_sb, qT_sb, 1)
                pro += [pro_vproj(("v", 3), st) for st in range(24, 28)]
                pro.append(pro_load(xqT, ("q", 2)))
                pro += proj_units(("q", 2), wq_sb, bq_sb, qT_sb, 2)
                pro += [pro_vproj(("v", 3), st) for st in range(28, 32)]
                pro.append(pro_load(xqT, ("q", 3)))
                pro += proj_units(("q", 3), wq_sb, bq_sb, qT_sb, 3)

                # ---------------- slot stream -------------------------------
                bg = []

                def bg_transpose_pair(ctxb2, j, qs):
                    # one PE transpose moves both h0 (partitions 0:64) and
                    # h1 (64:128) of plane 0 in a single [128,128] tile
                    def u():
                        tp = ps_c.tile([128, 512], f32, tag="big", name="tp")
                        tpv = tp[:, 0:64].bitcast(bf16)
                        nc.tensor.transpose(tpv, ctxb2[:, j, :, :], id_sb[:])
                        nc.scalar.copy(ctxT_sb[:, 0, qs], tpv)
                    return u

                def bg_transpose_h2(ctxb, j, qs):
                    def u():
                        tp = ps_c.tile([128, 512], f32, tag="big", name="tp")
                        tpv = tp[0:64, 0:64].bitcast(bf16)
                        nc.tensor.transpose(tpv, ctxb[:, j, :], id_sb[:])
                        nc.scalar.copy(ctxT_sb[0:64, 1, qs], tpv)
                    return u

                def bg_outproj(et, qn, drain=False):
                    def u():
                        es = slice(et * 128, (et + 1) * 128)
                        qs = slice(qn * 512, (qn + 1) * 512)
                        if drain:
                            # scores pool is idle during the final drain; use
                            # it for a deeper out-projection pipeline
                            po3t = ps_s.tile([128, QT], f32, tag="s",
                                             name="po3t")
                            po3 = po3t[:, 0:512]
                        else:
                            po3 = ps_c.tile([128, 512], f32, tag="big",
                                            name="po3")
                        nc.tensor.matmul(
                            po3[:], wo_sb[:, 0, es], ctxT_sb[:, 0, qs],
                            start=True, stop=False,
                        )
                        nc.tensor.matmul(
                            po3[:], wo_sb[0:64, 1, es], ctxT_sb[0:64, 1, qs],
                            start=False, stop=True,
                        )
                        ot = out_pool.tile([128, 512], f32, tag="ot", name="ot")
                        if et % 2 == 0:
                            nc.scalar.copy(ot[:], po3[:])
                        else:
                            nc.vector.tensor_copy(ot[:], po3[:])
                        nc.sync.dma_start(outT[es, qs], ot[:])
                    return u

                def emit_scores_exp(j, blk, h, kt):
                    pr, po = h // 2, 64 * (h % 2)
                    lhsT_k = kT_sb[po:po + 64, pr, kt * 128:(kt + 1) * 128]
                    pss = ps_s.tile([128, QT], f32, tag="s", name="pss")
                    expt = expt_pool.tile([128, QT], bf16, tag="expt",
                                          name="expt")
                    for n in range(QT // 512):
                        ns = slice(n * 512, (n + 1) * 512)
                        qs = slice(blk * QT + n * 512, blk * QT + (n + 1) * 512)
                        nc.tensor.matmul(
                            pss[:, ns], lhsT_k, qT_sb[po:po + 64, pr, qs],
                            start=True, stop=True,
                        )
                    # DVE takes every other tile away from boundaries (its
                    # norm burst lands in the first slots of a sweep)
                    if kt >= 4 and kt % 2 == 0:
                        nc.vector.tensor_scalar(
                            expt[:].bitcast(i16), pss[:],
                            float(SCHA), float(SCHB), MUL, ADD,
                        )
                    else:
                        nc.scalar.activation(
                            expt[:], pss[:], EXP, scale=0.125,
                        )
                    return expt

                state = {}

                def emit_ctx(blk, h, kt, expt):
                    if kt == 0:
                        ta = ps_c.tile([128, 512], f32, tag="big", name="psca")
                        tb = ps_c.tile([128, 512], f32, tag="big", name="pscb")
                        state["psc"] = (
                            ta[:, 0:4 * (D + 1)].rearrange(
                                "p (a b) -> p a b", a=4),
                            tb[:, 0:4 * (D + 1)].rearrange(
                                "p (a b) -> p a b", a=4),
                        )
                    psca, pscb = state["psc"]
                    first, last = (kt == 0), (kt == NKT - 1)
                    for jj in range(8):
                        pv = psca if jj < 4 else pscb
                        cc = jj % 4
                        # one accumulation group per psum bank: only the
                        # bank's first matmul starts (lazy-zeroing the whole
                        # bank), only its final matmul stops
                        nc.tensor.matmul(
                            pv[:, cc, :], expt[:, jj * 128:(jj + 1) * 128],
                            v_sb[:, kt, h, :],
                            start=(first and cc == 0), stop=(last and cc == 3),
                        )
                    if last:
                        emit_norm(blk, h, psca, pscb)

                def emit_norm(blk, h, psca, pscb):
                    rc = nrm_pool.tile([128, 8], f32, tag="rc", name="rc")
                    nc.vector.reciprocal(rc[:, 0:4], psca[:, :, D])
                    nc.vector.reciprocal(rc[:, 4:8], pscb[:, :, D])
                    if h == 0:
                        state["cb2"] = ctxb_pool.tile(
                            [128, 8, 2, D], bf16, tag="cb2", name="cb2")
                    if h < 2:
                        dsts = [state["cb2"][:, j, h, :] for j in range(8)]
                    else:
                        ctxb = ctxb_pool.tile([128, 8, D], bf16, tag="ctxb",
                                              name="ctxb")
                        dsts = [ctxb[:, j, :] for j in range(8)]
                    for j in range(8):
                        pv = psca if j < 4 else pscb
                        nc.vector.tensor_scalar_mul(
                            dsts[j], pv[:, j % 4, 0:D], rc[:, j:j + 1])
                    for j in range(8):
                        qs = slice(blk * QT + j * 128, blk * QT + (j + 1) * 128)
                        if h == 1:
                            bg.append(bg_transpose_pair(
                                state["cb2"], j, qs))
                        elif h == 2:
                            bg.append(bg_transpose_h2(ctxb, j, qs))
                    if h == HL - 1:
                        for et in range(6):
                            for n in range(QT // 512):
                                bg.append(bg_outproj(et, blk * 2 + n,
                                                     drain=(blk == NSQ - 1)))

                slots = [(blk, h, kt)
                         for blk in range(NSQ)
                         for h in range(HL)
                         for kt in range(NKT)]
                pending = []

                def pop_one():
                    (s2, e2) = pending.pop(0)
                    emit_ctx(*s2, e2)

                for j, slot in enumerate(slots):
                    expt = emit_scores_exp(j, *slot)
                    pending.append((slot, expt))
                    trail_eff = TRAIL if j < len(slots) - 12 else 2
                    for _ in range(3):
                        if not pending:
                            break
                        nblk, nh, nkt = pending[0][0]
                        # head boundaries leave GAP slots (block boundaries a
                        # bit more) so the psum pool turns over (norm +
                        # transposes + outproj tiles) before the next group
                        need = trail_eff
                        if nkt == 0:
                            need += GAPS[nh]
                        if len(pending) > need:
                            pop_one()
                        else:
                            break
                    for _ in range(3 if j < 40 else 2):
                        if pro:
                            pro.pop(0)()
                    drain = 3 if len(bg) > 10 else 2
                    for _ in range(drain):
                        if bg:
                            bg.pop(0)()
                while pending:
                    pop_one()
                    for _ in range(3):
                        if bg:
                            bg.pop(0)()
                while bg:
                    bg.pop(0)()
                assert not pro

    nc.compile()
    return nc


def _get_nc():
    with _lock:
        if "nc" not in _compiled:
            _compiled["nc"] = _build()
        return _compiled["nc"]


def _prep_in_maps(query, key, value, prompt, Wq, bq, Wk, bk, Wv, bv, Wo, bo):
    f32 = np.float32
    qT = [np.ascontiguousarray(query[b].T).astype(BF16) for b in range(B)]
    kT = [np.ascontiguousarray(key[b].T).astype(BF16) for b in range(B)]
    vT = [np.ascontiguousarray(value[b].T).astype(BF16) for b in range(B)]
    ident = np.eye(128, dtype=BF16)
    in_maps = []
    for core in range(NCORES):
        b, g = core // NG, core % NG
        cs = slice(g * CL, (g + 1) * CL)
        kp = np.zeros((128, 2, 128), BF16)
        vp = np.zeros((128, HL, D + 1), BF16)
        vp[0:PP, :, D] = 1.0
        for h in range(HL):
            gh = g * HL + h
            kp[64 * (h % 2):64 * (h % 2) + 64, h // 2, 0:PP] = (
                prompt[b, 0, :, gh, :].T.astype(BF16))
            vp[0:PP, h, 0:D] = prompt[b, 1, :, gh, :].astype(BF16)
        in_maps.append({
            "xqT": qT[b], "xkT": kT[b], "xvT": vT[b],
            "wqT": np.ascontiguousarray(Wq[cs, :].T).astype(BF16),
            "wkT": np.ascontiguousarray(Wk[cs, :].T).astype(BF16),
            "wvT": np.ascontiguousarray(Wv[cs, :].T).astype(BF16),
            "woT": np.ascontiguousarray(Wo[:, cs].T).astype(BF16),
            "bq": np.ascontiguousarray(bq[cs]).astype(f32).reshape(CL, 1),
            "bk": np.ascontiguousarray(bk[cs]).astype(f32).reshape(CL, 1),
            "bv": np.ascontiguousarray(bv[cs]).astype(f32).reshape(1, CL),
            "kpre": kp, "vpre": vp, "ident": ident,
        })
    return in_maps


def _combine(results, bo):
    out = np.empty((B, S, E), np.float32)
    for b in range(B):
        acc = results[b * NG]["outT"].astype(np.float32)
        for g in range(1, NG):
            acc = acc + results[b * NG + g]["outT"]
        out[b] = acc.T
    if bo is not None and np.any(bo):
        out += np.asarray(bo, np.float32)
    return out


def run(inputs, trace=False):
    """Returns (output, exec_time_ns or None)."""
    from concourse import bass_utils

    nc = _get_nc()
    in_maps = _prep_in_maps(**{k: np.asarray(v) for k, v in inputs.items()})
    bo = np.asarray(inputs["bo"])
    res = bass_utils.run_bass_kernel_spmd(
        nc, in_maps, core_ids=list(range(NCORES)), trace=trace,
    )
    return _combine(res.results, bo), res.exec_time_ns


def kernel(**inputs):
    out, _ = run(inputs)
    return out


# revision 22
# speedup vs baseline: 1.2008x; 1.0006x over previous
"""Multi-head attention (B=2,S=4096,E=768,H=12,D=64 + 16-token K/V prompt
prefix) on 8 Trainium2 NeuronCores.

Sharding: 2 batches x 4 head-groups (3 heads each). Each core computes QKV
projections for its 3 heads, full attention over its batch, and a partial
output projection (its 192 ctx channels); the host sums the 4 partials per
batch.

Key structure (v3):
- Uniform 33-k-tile pipeline: k-tile 32 holds the 16-token prompt prefix plus
  112 zero-padded keys whose v/ones-column are zero, so the pads contribute
  nothing to numerator or denominator.  No special-case prefix path.
- scoresT[k,q] psum from kT/qT slices (contraction D=64); exp is split across
  the Activation engine (exact, ~70% of tiles) and the Vector engine (~30%,
  Schraudolph int16/bf16-bitcast approximate exp: t = s*A+B -> int16 ->
  reinterpret as bf16 ~= e^s, max ~3.5% elementwise, ~1e-2 final L2).
  Triple-buffered scores psum decouples the PE->exp handshake.
- ctx computed in [q, d] orientation: lhsT = exp tile [128k, 128q], rhs =
  v_aug [128k, 65] (64 v dims + ones column) -> psum chunk [128q, 65]
  accumulated over the 33 k-tiles; the 65th column is the softmax
  denominator.  PSUM zero-region rule (one open accumulation group per 2KB
  bank): each [128, 4, 65] half-group lives in one bank with a single
  start (first matmul) / stop (last matmul).
- Normalized ctx [q, d] transposed back to [d, q] with PE-array transposes
  (identity matmul); transposes and out-projection tiles borrow the ctx psum
  bank pool between head groups.
- All Q/K/V projections run in a PE-only prologue (Act idles there but is
  not the bottleneck).
"""

import sys
import threading

import numpy as np

if "/opt/trn_rl_repo" not in sys.path:
    sys.path.insert(0, "/opt/trn_rl_repo")

import ml_dtypes

BF16 = ml_dtypes.bfloat16

B, S, E, H, D, PP = 2, 4096, 768, 12, 64, 16
NCORES = 8
NG = 4            # head-groups (tensor parallel)
HL = H // NG      # 3 local heads
CL = HL * D       # 192 local channels
NKT = 33          # 32 key tiles + 1 prefix/pad tile
SKVP = NKT * 128  # padded K/V length 4224
QT = 1024         # q block width
NSQ = S // QT     # 4
TRAIL = 6         # ctx matmuls trail scores by this many slots
# extra trail after each boundary so the shared psum pool turns over
# (before h1: norm only; before h2: norm + paired transposes; before a new
# block: norm + transposes + out-projection tiles)
GAPS = {0: 14, 1: 4, 2: 7}

# Schraudolph constants for bf16 bitcast exp: t = int16(s*SCHA + SCHB),
# bitcast(t) ~= e^s.  SCHA folds the 1/sqrt(D) logit scale.  SCHB centers
# the linear-mantissa interpolation error (C=0.0436 -> +-3% elementwise).
SCHA = (128.0 / np.log(2.0)) * 0.125
SCHB = 127.0 * 128.0 - 128.0 * 0.0436

_lock = threading.Lock()
_compiled = {}


def _build():
    import concourse.bass as bass  # noqa: F401
    import concourse.mybir as mybir
    import concourse.tile as tile
    from concourse import bacc

    f32 = mybir.dt.float32
    bf16 = mybir.dt.bfloat16
    i16 = mybir.dt.int16
    EXP = mybir.ActivationFunctionType.Exp
    MUL = mybir.AluOpType.mult
    ADD = mybir.AluOpType.add

    nc = bacc.Bacc("TRN2", target_bir_lowering=False, debug=False)

    xqT = nc.dram_tensor("xqT", [E, S], bf16, kind="ExternalInput").ap()
    xkT = nc.dram_tensor("xkT", [E, S], bf16, kind="ExternalInput").ap()
    xvT = nc.dram_tensor("xvT", [E, S], bf16, kind="ExternalInput").ap()
    wqT = nc.dram_tensor("wqT", [E, CL], bf16, kind="ExternalInput").ap()
    wkT = nc.dram_tensor("wkT", [E, CL], bf16, kind="ExternalInput").ap()
    wvT = nc.dram_tensor("wvT", [E, CL], bf16, kind="ExternalInput").ap()
    woT = nc.dram_tensor("woT", [CL, E], bf16, kind="ExternalInput").ap()
    bq = nc.dram_tensor("bq", [CL, 1], f32, kind="ExternalInput").ap()
    bk = nc.dram_tensor("bk", [CL, 1], f32, kind="ExternalInput").ap()
    bv = nc.dram_tensor("bv", [1, CL], f32, kind="ExternalInput").ap()
    kpre = nc.dram_tensor("kpre", [128, 2, 128], bf16, kind="ExternalInput").ap()
    vpre = nc.dram_tensor("vpre", [128, HL, D + 1], bf16,
                          kind="ExternalInput").ap()
    ident = nc.dram_tensor("ident", [128, 128], bf16, kind="ExternalInput").ap()
    outT = nc.dram_tensor("outT", [E, S], f32, kind="ExternalOutput").ap()

    with tile.TileContext(nc) as tc:
        with tc.tile_pool(name="persist", bufs=1) as pers:
            # q-projection weights/bias first: they gate the first matmuls
            wq_sb = pers.tile([128, 6, CL], bf16)
            nc.sync.dma_start(wq_sb[:], wqT.rearrange("(t p) c -> p t c", p=128))
            bq_sb = pers.tile([128, 2], f32)
            nc.sync.dma_start(bq_sb[:, 0:1], bq[0:128, :])
            nc.sync.dma_start(bq_sb[0:64, 1:2], bq[128:CL, :])

            wk_sb = pers.tile([128, 6, CL], bf16)
            wv_sb = pers.tile([128, 6, CL], bf16)
            wo_sb = pers.tile([128, 2, E], bf16)
            bk_sb = pers.tile([128, 2], f32)
            bvb_sb = pers.tile([128, CL], f32)
            id_sb = pers.tile([128, 128], bf16)

            # activations (bf16)
            qT_sb = pers.tile([128, 2, S], bf16)
            kT_sb = pers.tile([128, 2, SKVP], bf16)
            v_sb = pers.tile([128, NKT, HL, D + 1], bf16)
            ctxT_sb = pers.tile([128, 2, S], bf16)

            with (
                tc.tile_pool(name="ps_s", bufs=3, space="PSUM") as ps_s,
                tc.tile_pool(name="ps_c", bufs=2, space="PSUM") as ps_c,
                tc.tile_pool(name="x_pool", bufs=4) as x_pool,
                tc.tile_pool(name="expt_pool", bufs=TRAIL + 18) as expt_pool,
                tc.tile_pool(name="ctxb_pool", bufs=2) as ctxb_pool,
                tc.tile_pool(name="nrm_pool", bufs=2) as nrm_pool,
                tc.tile_pool(name="out_pool", bufs=6) as out_pool,
            ):
                # ------------- prologue: all Q/K/V projections --------------
                def load_x(src, blk, split=False):
                    xt = x_pool.tile([128, 6, QT], bf16, tag="x", name="x")
                    view = src.rearrange("(t p) s -> p t s", p=128)[
                        :, :, blk * QT:(blk + 1) * QT]
                    if split:
                        # one DMA per contraction chunk so the first matmul
                        # starts as soon as chunk 0 lands
                        for ech in range(6):
                            nc.sync.dma_start(xt[:, ech, :], view[:, ech, :])
                    else:
                        nc.sync.dma_start(xt[:], view)
                    return xt

                def proj_block(xt, wsb, bsb, dst, blk):
                    p0 = ps_s.tile([128, QT], f32, tag="s", name="p0")
                    p1t = ps_s.tile([128, QT], f32, tag="s", name="p1t")
                    for ech in range(6):
                        for n in range(QT // 512):
                            ns = slice(n * 512, (n + 1) * 512)
                            nc.tensor.matmul(
                                p0[:, ns], wsb[:, ech, 0:128], xt[:, ech, ns],
                                start=(ech == 0), stop=(ech == 5),
                            )
                            nc.tensor.matmul(
                                p1t[0:64, ns], wsb[:, ech, 128:CL],
                                xt[:, ech, ns],
                                start=(ech == 0), stop=(ech == 5),
                            )
                    ds = slice(blk * QT, (blk + 1) * QT)
                    nc.vector.tensor_scalar_add(
                        dst[:, 0, ds], p0[:], bsb[:, 0:1])
                    nc.vector.tensor_scalar_add(
                        dst[0:64, 1, ds], p1t[0:64, :], bsb[0:64, 1:2])

                def emit_vproj(xt, st):
                    stl = st % 8
                    pvt = ps_s.tile([128, QT], f32, tag="s", name="pvt")
                    pv = pvt[:, 0:CL]
                    for ech in range(6):
                        nc.tensor.matmul(
                            pv,
                            xt[:, ech, stl * 128:(stl + 1) * 128],
                            wv_sb[:, ech, :],
                            start=(ech == 0), stop=(ech == 5),
                        )
                    nc.vector.tensor_add(
                        v_sb[:, st, :, 0:D],
                        pv.rearrange("p (h d) -> p h d", h=HL),
                        bvb_sb[:].rearrange("p (h d) -> p h d", h=HL),
                    )

                xq0 = load_x(xqT, 0, split=True)
                # critical-path DMAs for the K projection next
                nc.sync.dma_start(
                    wk_sb[:], wkT.rearrange("(t p) c -> p t c", p=128))
                nc.sync.dma_start(bk_sb[:, 0:1], bk[0:128, :])
                nc.sync.dma_start(bk_sb[0:64, 1:2], bk[128:CL, :])
                xk0 = load_x(xkT, 0)
                nc.sync.dma_start(
                    wv_sb[:], wvT.rearrange("(t p) c -> p t c", p=128))
                nc.sync.dma_start(bvb_sb[:], bv.to_broadcast((128, CL)))
                xv0 = load_x(xvT, 0)
                proj_block(xq0, wq_sb, bq_sb, qT_sb, 0)

                # remaining non-critical inputs
                nc.sync.dma_start(kT_sb[:, :, S:SKVP], kpre[:])
                nc.sync.dma_start(v_sb[:, NKT - 1, :, :], vpre[:])
                nc.sync.dma_start(wo_sb[:, 0, :], woT[0:128, :])
                nc.sync.dma_start(wo_sb[0:64, 1, :], woT[128:CL, :])
                nc.sync.dma_start(id_sb[:], ident[:])
                nc.vector.memset(v_sb[:, 0:NKT - 1, :, D:D + 1], 1.0)

                # serial pre-stream work: K block 0 (gates the first scores)
                # and V tiles 0..7 (gate the first ctx pops)
                proj_block(xk0, wk_sb, bk_sb, kT_sb, 0)
                for st in range(8):
                    emit_vproj(xv0, st)

                # The remaining projections (K1-3, V8-31, Q1-3) drain into the
                # early slot stream as small units (~2 per slot) so the exp
                # engines start ~50us earlier.  x-tile loads are embedded with
                # enough lead; deadlines: K block b by slot 8b, V tile st by
                # slot st+TRAIL, Q block b by slot 99b.
                pro = []
                xts = {}

                def pro_load(src, key):
                    def u():
                        xts[key] = load_x(src, key[1])
                    return u

                def pro_proj_mm(key, wsb, dst_half, n):
                    def u():
                        if n == 0:
                            xts[(key, "p", dst_half)] = ps_s.tile(
                                [128, QT], f32, tag="s", name="pp")
                        pt = xts[(key, "p", dst_half)]
                        rows = 128 if dst_half == 0 else 64
                        wc = slice(0, 128) if dst_half == 0 else slice(128, CL)
                        ns = slice(n * 512, (n + 1) * 512)
                        for ech in range(6):
                            nc.tensor.matmul(
                                pt[0:rows, ns], wsb[:, ech, wc],
                                xts[key][:, ech, ns],
                                start=(ech == 0), stop=(ech == 5),
                            )
                    return u

                def pro_proj_evac(key, bsb, dst, blk, dst_half):
                    def u():
                        pt = xts.pop((key, "p", dst_half))
                        ds = slice(blk * QT, (blk + 1) * QT)
                        if dst_half == 0:
                            nc.vector.tensor_scalar_add(
                                dst[:, 0, ds], pt[:], bsb[:, 0:1])
                        else:
                            nc.vector.tensor_scalar_add(
                                dst[0:64, 1, ds], pt[0:64, :],
                                bsb[0:64, 1:2])
                    return u

                def pro_vproj(key, st):
                    def u():
                        emit_vproj(xts[key], st)
                    return u

                def proj_units(key, wsb, bsb, dst, blk):
                    return [
                        pro_proj_mm(key, wsb, 0, 0), pro_proj_mm(key, wsb, 0, 1),
                        pro_proj_evac(key, bsb, dst, blk, 0),
                        pro_proj_mm(key, wsb, 1, 0), pro_proj_mm(key, wsb, 1, 1),
                        pro_proj_evac(key, bsb, dst, blk, 1),
                    ]

                # prefetch the first stream-phase loads
                xts[("k", 1)] = load_x(xkT, 1)
                xts[("k", 2)] = load_x(xkT, 2)
                xts[("v", 1)] = load_x(xvT, 1)
                xts[("v", 0)] = xv0

                pro += proj_units(("k", 1), wk_sb, bk_sb, kT_sb, 1)
                pro += [pro_vproj(("v", 1), st) for st in range(8, 12)]
                pro.append(pro_load(xvT, ("v", 2)))
                pro += proj_units(("k", 2), wk_sb, bk_sb, kT_sb, 2)
                pro += [pro_vproj(("v", 1), st) for st in range(12, 16)]
                pro.append(pro_load(xkT, ("k", 3)))
                pro += [pro_vproj(("v", 2), st) for st in range(16, 20)]
                pro += proj_units(("k", 3), wk_sb, bk_sb, kT_sb, 3)
                pro.append(pro_load(xqT, ("q", 1)))
                pro += [pro_vproj(("v", 2), st) for st in range(20, 24)]
                pro.append(pro_load(xvT, ("v", 3)))
                pro += proj_units(("q", 1), wq_sb, bq_sb, qT_sb, 1)
                pro += [pro_vproj(("v", 3), st) for st in range(24, 28)]
                pro.append(pro_load(xqT, ("q", 2)))
                pro += proj_units(("q", 2), wq_sb, bq_sb, qT_sb, 2)
                pro += [pro_vproj(("v", 3), st) for st in range(28, 32)]
                pro.append(pro_load(xqT, ("q", 3)))
                pro += proj_units(("q", 3), wq_sb, bq_sb, qT_sb, 3)

                # ---------------- slot stream -------------------------------
                bg = []

                def bg_transpose_pair(ctxb2, j, qs):
                    # one PE transpose moves both h0 (partitions 0:64) and
                    # h1 (64:128) of plane 0 in a single [128,128] tile
                    def u():
                        tp = ps_c.tile([128, 512], f32, tag="big", name="tp")
                        tpv = tp[:, 0:64].bitcast(bf16)
                        nc.tensor.transpose(tpv, ctxb2[:, j, :, :], id_sb[:])
                        nc.scalar.copy(ctxT_sb[:, 0, qs], tpv)
                    return u

                def bg_transpose_h2(ctxb, j, qs):
                    def u():
                        tp = ps_c.tile([128, 512], f32, tag="big", name="tp")
                        tpv = tp[0:64, 0:64].bitcast(bf16)
                        nc.tensor.transpose(tpv, ctxb[:, j, :], id_sb[:])
                        nc.scalar.copy(ctxT_sb[0:64, 1, qs], tpv)
                    return u

                def bg_outproj(et, qn, drain=False):
                    def u():
                        es = slice(et * 128, (et + 1) * 128)
                        qs = slice(qn * 512, (qn + 1) * 512)
                        if drain:
                            # scores pool is idle during the final drain; use
                            # it for a deeper out-projection pipeline
                            po3t = ps_s.tile([128, QT], f32, tag="s",
                                             name="po3t")
                            po3 = po3t[:, 0:512]
                        else:
                            po3 = ps_c.tile([128, 512], f32, tag="big",
                                            name="po3")
                        nc.tensor.matmul(
                            po3[:], wo_sb[:, 0, es], ctxT_sb[:, 0, qs],
                            start=True, stop=False,
                        )
                        nc.tensor.matmul(
                            po3[:], wo_sb[0:64, 1, es], ctxT_sb[0:64, 1, qs],
                            start=False, stop=True,
                        )
                        ot = out_pool.tile([128, 512], f32, tag="ot", name="ot")
                        if et % 2 == 0:
                            nc.scalar.copy(ot[:], po3[:])
                        else:
                            nc.vector.tensor_copy(ot[:], po3[:])
                        nc.sync.dma_start(outT[es, qs], ot[:])
                    return u

                def emit_scores_exp(j, blk, h, kt):
                    pr, po = h // 2, 64 * (h % 2)
                    lhsT_k = kT_sb[po:po + 64, pr, kt * 128:(kt + 1) * 128]
                    pss = ps_s.tile([128, QT], f32, tag="s", name="pss")
                    expt = expt_pool.tile([128, QT], bf16, tag="expt",
                                          name="expt")
                    for n in range(QT // 512):
                        ns = slice(n * 512, (n + 1) * 512)
                        qs = slice(blk * QT + n * 512, blk * QT + (n + 1) * 512)
                        nc.tensor.matmul(
                            pss[:, ns], lhsT_k, qT_sb[po:po + 64, pr, qs],
                            start=True, stop=True,
                        )
                    # DVE takes every other tile away from boundaries (its
                    # norm burst lands in the first slots of a sweep)
                    if kt >= 4 and kt % 2 == 0:
                        nc.vector.tensor_scalar(
                            expt[:].bitcast(i16), pss[:],
                            float(SCHA), float(SCHB), MUL, ADD,
                        )
                    else:
                        nc.scalar.activation(
                            expt[:], pss[:], EXP, scale=0.125,
                        )
                    return expt

                state = {}

                def emit_ctx(blk, h, kt, expt):
                    if kt == 0:
                        ta = ps_c.tile([128, 512], f32, tag="big", name="psca")
                        tb = ps_c.tile([128, 512], f32, tag="big", name="pscb")
                        state["psc"] = (
                            ta[:, 0:4 * (D + 1)].rearrange(
                                "p (a b) -> p a b", a=4),
                            tb[:, 0:4 * (D + 1)].rearrange(
                                "p (a b) -> p a b", a=4),
                        )
                    psca, pscb = state["psc"]
                    first, last = (kt == 0), (kt == NKT - 1)
                    for jj in range(8):
                        pv = psca if jj < 4 else pscb
                        cc = jj % 4
                        # one accumulation group per psum bank: only the
                        # bank's first matmul starts (lazy-zeroing the whole
                        # bank), only its final matmul stops
                        nc.tensor.matmul(
                            pv[:, cc, :], expt[:, jj * 128:(jj + 1) * 128],
                            v_sb[:, kt, h, :],
                            start=(first and cc == 0), stop=(last and cc == 3),
                        )
                    if last:
                        emit_norm(blk, h, psca, pscb)

                def emit_norm(blk, h, psca, pscb):
                    rc = nrm_pool.tile([128, 8], f32, tag="rc", name="rc")
                    nc.vector.reciprocal(rc[:, 0:4], psca[:, :, D])
                    nc.vector.reciprocal(rc[:, 4:8], pscb[:, :, D])
                    if h == 0:
                        state["cb2"] = ctxb_pool.tile(
                            [128, 8, 2, D], bf16, tag="cb2", name="cb2")
                    if h < 2:
                        dsts = [state["cb2"][:, j, h, :] for j in range(8)]
                    else:
                        ctxb = ctxb_pool.tile([128, 8, D], bf16, tag="ctxb",
                                              name="ctxb")
                        dsts = [ctxb[:, j, :] for j in range(8)]
                    for j in range(8):
                        pv = psca if j < 4 else pscb
                        nc.vector.tensor_scalar_mul(
                            dsts[j], pv[:, j % 4, 0:D], rc[:, j:j + 1])
                    for j in range(8):
                        qs = slice(blk * QT + j * 128, blk * QT + (j + 1) * 128)
                        if h == 1:
                            bg.append(bg_transpose_pair(
                                state["cb2"], j, qs))
                        elif h == 2:
                            bg.append(bg_transpose_h2(ctxb, j, qs))
                    if h == HL - 1:
                        for et in range(6):
                            for n in range(QT // 512):
                                bg.append(bg_outproj(et, blk * 2 + n,
                                                     drain=(blk == NSQ - 1)))

                slots = [(blk, h, kt)
                         for blk in range(NSQ)
                         for h in range(HL)
                         for kt in range(NKT)]
                pending = []

                def pop_one():
                    (s2, e2) = pending.pop(0)
                    emit_ctx(*s2, e2)

                for j, slot in enumerate(slots):
                    expt = emit_scores_exp(j, *slot)
                    pending.append((slot, expt))
                    trail_eff = TRAIL if j < len(slots) - 12 else 2
                    for _ in range(3):
                        if not pending:
                            break
                        nblk, nh, nkt = pending[0][0]
                        # head boundaries leave GAP slots (block boundaries a
                        # bit more) so the psum pool turns over (norm +
                        # transposes + outproj tiles) before the next group
                        need = trail_eff
                        if nkt == 0:
                            need += GAPS[nh]
                        if len(pending) > need:
                            pop_one()
                        else:
                            break
                    for _ in range(3 if j < 40 else 2):
                        if pro:
                            pro.pop(0)()
                    drain = 3 if len(bg) > 10 else 2
                    for _ in range(drain):
                        if bg:
                            bg.pop(0)()
                while pending:
                    pop_one()
                    for _ in range(3):
                        if bg:
                            bg.pop(0)()
                while bg:
                    bg.pop(0)()
                assert not pro

    nc.compile()
    return nc


def _get_nc():
    with _lock:
        if "nc" not in _compiled:
            _compiled["nc"] = _build()
        return _compiled["nc"]


def _prep_in_maps(query, key, value, prompt, Wq, bq, Wk, bk, Wv, bv, Wo, bo):
    f32 = np.float32
    qT = [np.ascontiguousarray(query[b].T).astype(BF16) for b in range(B)]
    kT = [np.ascontiguousarray(key[b].T).astype(BF16) for b in range(B)]
    vT = [np.ascontiguousarray(value[b].T).astype(BF16) for b in range(B)]
    ident = np.eye(128, dtype=BF16)
    in_maps = []
    for core in range(NCORES):
        b, g = core // NG, core % NG
        cs = slice(g * CL, (g + 1) * CL)
        kp = np.zeros((128, 2, 128), BF16)
        vp = np.zeros((128, HL, D + 1), BF16)
        vp[0:PP, :, D] = 1.0
        for h in range(HL):
            gh = g * HL + h
            kp[64 * (h % 2):64 * (h % 2) + 64, h // 2, 0:PP] = (
                prompt[b, 0, :, gh, :].T.astype(BF16))
            vp[0:PP, h, 0:D] = prompt[b, 1, :, gh, :].astype(BF16)
        in_maps.append({
            "xqT": qT[b], "xkT": kT[b], "xvT": vT[b],
            "wqT": np.ascontiguousarray(Wq[cs, :].T).astype(BF16),
            "wkT": np.ascontiguousarray(Wk[cs, :].T).astype(BF16),
            "wvT": np.ascontiguousarray(Wv[cs, :].T).astype(BF16),
            "woT": np.ascontiguousarray(Wo[:, cs].T).astype(BF16),
            "bq": np.ascontiguousarray(bq[cs]).astype(f32).reshape(CL, 1),
            "bk": np.ascontiguousarray(bk[cs]).astype(f32).reshape(CL, 1),
            "bv": np.ascontiguousarray(bv[cs]).astype(f32).reshape(1, CL),
            "kpre": kp, "vpre": vp, "ident": ident,
        })
    return in_maps


def _combine(results, bo):
    out = np.empty((B, S, E), np.float32)
    for b in range(B):
        acc = results[b * NG]["outT"].astype(np.float32)
        for g in range(1, NG):
            acc = acc + results[b * NG + g]["outT"]
        out[b] = acc.T
    if bo is not None and np.any(bo):
        out += np.asarray(bo, np.float32)
    return out


def run(inputs, trace=False):
    """Returns (output, exec_time_ns or None)."""
    from concourse import bass_utils

    nc = _get_nc()
    in_maps = _prep_in_maps(**{k: np.asarray(v) for k, v in inputs.items()})
    bo = np.asarray(inputs["bo"])
    res = bass_utils.run_bass_kernel_spmd(
        nc, in_maps, core_ids=list(range(NCORES)), trace=trace,
    )
    return _combine(res.results, bo), res.exec_time_ns


def kernel(**inputs):
    out, _ = run(inputs)
    return out
